# revision 1
# baseline (speedup 1.0000x reference)
"""Trainium2 Bass kernel for the ragged 2-layer GRU problem (nn_DeepIOFeat11).

Contract: kernel(**inputs) takes FULL numpy inputs, returns FULL [N, H] f32 output.

Strategy (see NOTES.md):
- Sort sequences by length DESC, deal round-robin across 8 cores (pure data parallel).
- Per core: one mega-"chunk"; transposed layout: 21 groups of 6 hidden rows on
  partitions (126 rows + a ones-row), sequences along the free dim, column-major
  rank -> (g = r % 21, col = r // 21) so active columns form a shrinking prefix.
- Per (layer, step): 6 small matmuls (block-diag weights, K=127, M=126, N=W_t)
  compute r/z gates (x-side + h-side accumulated in PSUM) and the two n-gate
  halves; ACT does sigmoid/tanh; DVE/GPSIMD do the gate algebra; the update is
  frozen per-sequence once t >= len via z'' = max(z, [len <= t]) on the top layer.
- W_t (active column count at step t) is baked at trace time from the actual
  lengths; all engines slice [:, :W_t].
"""

import math

import numpy as np

import concourse.bacc as bacc
import concourse.bass as bass
import concourse.mybir as mybir
import concourse.tile as tile
from concourse.bass_utils import run_bass_kernel_spmd

NC = 8          # cores
G = 21          # groups per core
HP = 6          # hidden size
P = G * HP      # 126 data partitions
KP = P + 1      # +1 ones row

F32 = mybir.dt.float32
AF = mybir.ActivationFunctionType
ALU = mybir.AluOpType


def _plan(lengths):
    """Sort desc, deal round-robin. Returns per-core rank->orig index and W schedule."""
    n = lengths.shape[0]
    order = np.argsort(-lengths, kind="stable")
    percore = [order[c::NC] for c in range(NC)]          # each desc-sorted
    s = max(len(pc) for pc in percore)
    w0 = math.ceil(s / G)
    t_max = int(lengths.max())
    # W_t = max over cores of ceil(cnt(len > t) / G)
    wts = []
    for t in range(t_max):
        w = 0
        for pc in percore:
            cnt = int(np.count_nonzero(lengths[pc] > t))
            w = max(w, math.ceil(cnt / G))
        wts.append(max(w, 1))
    return percore, w0, wts


def _build_lhst(W_ih, W_hh, b_ih, b_hh, l):
    """12 block-diag lhsT matrices -> dict[(side, gate)] of [KP, P] f32."""
    out = {}
    for side in ("x", "h"):
        Wm = W_ih[l] if side == "x" else W_hh[l]          # [18, 6]
        for qi, q in enumerate(("r", "z", "n")):
            m = np.zeros((KP, P), np.float32)
            blk = Wm[qi * HP:(qi + 1) * HP, :]           # [6(out j), 6(in k)]
            for g in range(G):
                m[g * HP:(g + 1) * HP, g * HP:(g + 1) * HP] = blk.T  # [k, j]
            if side == "x":
                bias = b_ih[l][qi * HP:(qi + 1) * HP].copy()
                if q != "n":
                    bias = bias + b_hh[l][qi * HP:(qi + 1) * HP]
            else:
                bias = (b_hh[l][qi * HP:(qi + 1) * HP]
                        if q == "n" else np.zeros(HP, np.float32))
            for g in range(G):
                m[P, g * HP:(g + 1) * HP] = bias
            out[(side, q)] = m
    return out


def _build_program(t_steps, w0, wts, n_dma_slices):
    """Trace the Bass program. Shapes depend only on (t_steps, w0, wts)."""
    nc = bacc.Bacc(None, target_bir_lowering=False)
    x_dram = nc.declare_dram_parameter("x_sb", [KP, t_steps * w0], F32, isOutput=False)
    len_dram = nc.declare_dram_parameter("len_t", [P, w0], F32, isOutput=False)
    lw_dram = nc.declare_dram_parameter("w_all", [KP, 12 * P], F32, isOutput=False)
    hinit_dram = nc.declare_dram_parameter("h_init", [KP, w0], F32, isOutput=False)
    out_dram = nc.declare_dram_parameter("out", [P, w0], F32, isOutput=True)

    with tile.TileContext(nc) as tc:
        with (
            tc.tile_pool(name="persist", bufs=1) as pp,
            tc.tile_pool(name="work", bufs=3) as wp,
            tc.tile_pool(name="psum", bufs=2, space=bass.MemorySpace.PSUM) as psp,
        ):
            x_sb = pp.tile([KP, t_steps * w0], F32)
            len_sb = pp.tile([P, w0], F32)
            h = [pp.tile([KP, w0], F32, tag=f"h{i}", name=f"h{i}") for i in range(2)]
            lw_all = pp.tile([KP, 12 * P], F32)
            nc.sync.dma_start(lw_all[:], lw_dram[:])
            lw = {}
            for i, l in enumerate(range(2)):
                for j, side in enumerate(("x", "h")):
                    for k, q in enumerate(("r", "z", "n")):
                        idx = l * 6 + j * 3 + k
                        lw[(l, side, q)] = lw_all[:, idx * P:(idx + 1) * P]
            nc.sync.dma_start(len_sb[:], len_dram[:])
            # x DMA in slices so compute can start early
            sl = math.ceil(t_steps / n_dma_slices)
            for i in range(n_dma_slices):
                a, b = i * sl * w0, min(t_steps, (i + 1) * sl) * w0
                if a < b:
                    nc.sync.dma_start(x_sb[:, a:b], x_dram[:, a:b])
            for l in range(2):
                nc.sync.dma_start(h[l][:], hinit_dram[:])

            for t in range(t_steps):
                w = wts[t]
                for l in range(2):
                    rhs_x = (x_sb[:, t * w0:t * w0 + w] if l == 0
                             else h[0][:, 0:w])
                    rhs_h = h[l][:, 0:w]
                    ps = {q: psp.tile([P, w0], F32, tag=f"ps_{q}", name=f"ps_{q}")
                          for q in ("r", "z", "xn", "hn")}
                    for q in ("r", "z"):
                        nc.tensor.matmul(ps[q][:, 0:w], lw[(l, "x", q)][:],
                                         rhs_x, start=True, stop=False)
                        nc.tensor.matmul(ps[q][:, 0:w], lw[(l, "h", q)][:],
                                         rhs_h, start=False, stop=True)
                    nc.tensor.matmul(ps["xn"][:, 0:w], lw[(l, "x", "n")][:],
                                     rhs_x, start=True, stop=True)
                    nc.tensor.matmul(ps["hn"][:, 0:w], lw[(l, "h", "n")][:],
                                     rhs_h, start=True, stop=True)

                    r_sb = wp.tile([P, w0], F32, tag="r_sb")
                    z_sb = wp.tile([P, w0], F32, tag="z_sb")
                    n_sb = wp.tile([P, w0], F32, tag="n_sb")
                    t1 = wp.tile([P, w0], F32, tag="t1")
                    nc.scalar.activation(r_sb[:, 0:w], ps["r"][:, 0:w], AF.Sigmoid)
                    nc.scalar.activation(z_sb[:, 0:w], ps["z"][:, 0:w], AF.Sigmoid)
                    nc.vector.tensor_mul(t1[:, 0:w], r_sb[:, 0:w], ps["hn"][:, 0:w])
                    nc.vector.tensor_add(t1[:, 0:w], t1[:, 0:w], ps["xn"][:, 0:w])
                    nc.scalar.activation(n_sb[:, 0:w], t1[:, 0:w], AF.Tanh)
                    if l == 1:
                        m01 = wp.tile([P, w0], F32, tag="m01")
                        nc.vector.tensor_scalar(m01[:, 0:w], len_sb[:, 0:w],
                                                float(t), None, ALU.is_le)
                        nc.vector.tensor_max(z_sb[:, 0:w], z_sb[:, 0:w],
                                             m01[:, 0:w])
                    d = wp.tile([P, w0], F32, tag="d")
                    e = wp.tile([P, w0], F32, tag="e")
                    nc.gpsimd.tensor_sub(d[:, 0:w], h[l][0:P, 0:w], n_sb[:, 0:w])
                    nc.gpsimd.tensor_mul(e[:, 0:w], z_sb[:, 0:w], d[:, 0:w])
                    nc.vector.tensor_add(h[l][0:P, 0:w], n_sb[:, 0:w], e[:, 0:w])

            nc.sync.dma_start(out_dram[:], h[1][0:P, :])
    nc.compile()
    return nc


def kernel(x, lengths, W_ih, W_hh, b_ih, b_hh):
    out, _ = kernel_traced(x=x, lengths=lengths, W_ih=W_ih, W_hh=W_hh,
                           b_ih=b_ih, b_hh=b_hh, trace=False)
    return out


def kernel_traced(x, lengths, W_ih, W_hh, b_ih, b_hh, trace=False):
    x = np.ascontiguousarray(x, np.float32)
    lengths = np.ascontiguousarray(lengths)
    n, t_dim, i_dim = x.shape
    assert i_dim == HP
    percore, w0, wts = _plan(lengths)
    t_steps = len(wts)

    lhst = {}
    for l in range(2):
        for k, v in _build_lhst(np.asarray(W_ih, np.float32), np.asarray(W_hh, np.float32),
                                np.asarray(b_ih, np.float32), np.asarray(b_hh, np.float32),
                                l).items():
            lhst[(l,) + k] = v

    in_maps = []
    for c in range(NC):
        idx = percore[c]
        s = len(idx)
        # x_core grid: [KP, t_steps, w0]; row 6g+k, col c2 -> seq rank c2*G+g
        xg = np.zeros((KP, t_steps, w0), np.float32)
        xs = x[idx][:, :t_steps, :]                      # [s, t_steps, 6]
        pad = G * w0 - s
        if pad:
            xs = np.concatenate([xs, np.zeros((pad, t_steps, HP), np.float32)], 0)
        # rank r=(col*G+g) -> row block g, col
        xr = xs.reshape(w0, G, t_steps, HP)              # [col, g, t, k]
        xg[0:P] = xr.transpose(1, 3, 2, 0).reshape(P, t_steps, w0)
        xg[P] = 1.0
        lens = lengths[idx].astype(np.float32)
        if pad:
            lens = np.concatenate([lens, np.ones(pad, np.float32)])
        lg = np.repeat(lens.reshape(w0, G), HP, axis=1).T.copy()  # [P, w0] rows 6g+k
        hinit = np.zeros((KP, w0), np.float32)
        hinit[P] = 1.0
        w_all = np.zeros((KP, 12 * P), np.float32)
        for l in range(2):
            for j, side in enumerate(("x", "h")):
                for k, q in enumerate(("r", "z", "n")):
                    idx = l * 6 + j * 3 + k
                    w_all[:, idx * P:(idx + 1) * P] = lhst[(l, side, q)]
        m = {"x_sb": xg.reshape(KP, t_steps * w0), "len_t": lg, "h_init": hinit,
             "w_all": w_all}
        in_maps.append(m)

    nc = _build_program(t_steps, w0, wts, n_dma_slices=16)
    bkr = run_bass_kernel_spmd(nc, in_maps, list(range(NC)), trace=trace)
    res = bkr.results

    out = np.zeros((n, HP), np.float32)
    for c in range(NC):
        idx = percore[c]
        og = res[c]["out"]                               # [P, w0]
        # row 6g+k, col -> rank col*G+g
        vals = og.reshape(G, HP, w0).transpose(2, 0, 1).reshape(G * w0, HP)
        out[idx] = vals[:len(idx)]
    return out, bkr

